# revision 24
# baseline (speedup 1.0000x reference)
"""Trainium2 Bass kernel for hardest-positive/hardest-negative triplet mining
(Miner, cosine distance) over embeddings [8192, 128] with int labels [8192].

Strategy (8 NeuronCores, 1D data-parallel over anchor rows):
  host: normalize rows (fp32), stable-sort rows by label, transpose to
        enT [d=128, n=8192]. Each core c gets the column-rotated copy
        enT[:, (j + 1024c) mod 8192] so its 1024 anchors sit at rotated
        columns [0, 1024). After sorting, all same-label columns for the
        anchors of row-block b live in the static 512-wide window
        W(b) = [128b-192, 128b+320) (label runs are <=49 long), so label
        masking touches only 512 columns per block -- identical AP
        offsets on all 8 SPMD cores.
  device (per core, 8 row-blocks of 128 anchors):
        PE:  fp32 matmuls  dot = enT_blk.T @ enT_cols -> PSUM (window
             tiles first, then runs of 3 tiles into one 3-bank psum tile)
        ACT: copy psum -> slab [128, 8192] (one copy per psum tile)
        DVE: wneg/wpos = +-2^30 * (labcol ==/!= labrow) over the window;
             pslab = slab[W] + wpos  (pos candidates, masked, 512 wide)
             slab[W] += wneg         (neg candidates, masked in place)
             16-granular seg maxes -> first seg with seg==m (match scan),
             in-seg position via gpsimd indirect_copy gather (16-partition
             groups share indices, so each partition gathers its group's
             16 winning segs and a host-built additive mask kills the 15
             foreign slots), pos index via direct 512-wide match scan.
  host: triplet assembly, un-rotate / un-sort indices, dist = 1 -/+ dot.

Exactness: masked values only carry +-2^30 offsets (never touch valid
values), reductions pick actual element values bitwise, and the index
scans match the reduced value exactly, so results equal an fp32
reference up to the matmul's own rounding (measured ~8e-8 abs).
"""

from contextlib import ExitStack

import numpy as np

import concourse.bacc as bacc
import concourse.mybir as mybir
import concourse.tile as tile
from concourse.bass_utils import run_bass_kernel_spmd

N = 8192
D = 128
NCORES = 8
ROWS_PER_CORE = N // NCORES          # 1024
NBLK = ROWS_PER_CORE // 128          # 8 row-blocks per core
NT = N // 512                        # 16 column tiles of 512
G = 16                               # seg granularity for the neg index
NSEG = N // G                        # 512 segs per row
GRP = 2                              # non-window tiles per psum group
USE_FP16 = False                         # fp16 hi/lo split matmuls vs fp32
BIGM = float(2.0 ** 38) if USE_FP16 else float(2.0 ** 30)
SCALE = float(2.0 ** 14) if USE_FP16 else 1.0
SCALE2 = SCALE * SCALE
EPS = 1e-8

# tiles containing window cols, per row-block (masked via slab)
_WT = {0: (15, 0), 1: (15, 0), 2: (0, 1), 3: (0, 1),
       4: (0, 1), 5: (0, 1), 6: (1, 2), 7: (1, 2)}


def _wstart(b):
    return (128 * b - 192) % N


def _wpieces(b):
    """(src_start, length, dst_start) covering window cols in slab space."""
    w = _wstart(b)
    if w + 512 <= N:
        return [(w, 512, 0)]
    l0 = N - w
    return [(w, l0, 0), (0, 512 - l0, l0)]


def _groups(b):
    """Contiguous runs (len <= GRP) of non-window tiles."""
    wt = set(_WT[b])
    runs, cur = [], []
    for t in range(NT):
        if t in wt:
            if cur:
                runs.append(cur)
                cur = []
        else:
            cur.append(t)
            if len(cur) == GRP:
                runs.append(cur)
                cur = []
    if cur:
        runs.append(cur)
    return runs


def _wsubpieces(b):
    """Window pieces split at tile boundaries: (tile, off_in_tile, len, dst)."""
    out = []
    for (s, ln, d) in _wpieces(b):
        while ln > 0:
            t = s // 512
            off = s - 512 * t
            l = min(ln, 512 - off)
            out.append((t, off, l, d))
            s += l
            ln -= l
            d += l
    return out


def _wcomplement(b):
    """Non-window ranges of the two window tiles: (tile, off_in_tile, len)."""
    cover = {}
    for (t, off, l, d) in _wsubpieces(b):
        lo, hi = cover.get(t, (off, off + l))
        cover[t] = (min(lo, off), max(hi, off + l))
    comp = []
    for t in _WT[b]:
        lo, hi = cover.get(t, (0, 0))
        if lo > 0:
            comp.append((t, 0, lo))
        if hi < 512:
            comp.append((t, hi, 512 - hi))
    return comp


# process blocks so early input-DMA chunks unblock the first blocks; the
# final block uses a direct full-row scan instead of the gather
_BLK_ORDER = [2, 3, 4, 5, 6, 7, 0, 1]
_DIRECT_BLK = _BLK_ORDER[-1]


_NC = None


def _build():
    f32 = mybir.dt.float32
    u32 = mybir.dt.uint32
    u16 = mybir.dt.uint16
    AX = mybir.AxisListType
    OP = mybir.AluOpType

    nc = bacc.Bacc("TRN2", target_bir_lowering=False, debug=False,
                   num_devices=NCORES)

    f16 = mybir.dt.float16
    edt = f16 if USE_FP16 else f32
    enth_d = nc.dram_tensor("enth", [D, N], edt, kind="ExternalInput").ap()
    if USE_FP16:
        entl_d = nc.dram_tensor("entl", [D, N], edt, kind="ExternalInput").ap()
    labwin_d = nc.dram_tensor("labwin", [128, 512 * NBLK], f32,
                              kind="ExternalInput").ap()
    labrow_d = nc.dram_tensor("labrow", [128, NBLK], f32, kind="ExternalInput").ap()
    gmask_d = nc.dram_tensor("gmask", [128, 16 * G], f32, kind="ExternalInput").ap()

    negmax_d = nc.dram_tensor("negmax", [128, NBLK], f32, kind="ExternalOutput").ap()
    posmin_d = nc.dram_tensor("posmin", [128, NBLK], f32, kind="ExternalOutput").ap()
    negseg_d = nc.dram_tensor("negseg", [NBLK, 128, 8], u32, kind="ExternalOutput").ap()
    negoff_d = nc.dram_tensor("negoff", [NBLK, 128, 8], u32, kind="ExternalOutput").ap()
    posidx_d = nc.dram_tensor("posidx", [NBLK, 128, 8], u32, kind="ExternalOutput").ap()

    with tile.TileContext(nc) as tc, ExitStack() as ctx:
        consts = ctx.enter_context(tc.tile_pool(name="consts", bufs=1))
        slabs = ctx.enter_context(tc.tile_pool(name="slabs", bufs=2))
        pslabs = ctx.enter_context(tc.tile_pool(name="pslabs", bufs=2))
        masks = ctx.enter_context(tc.tile_pool(name="masks", bufs=4))
        segs = ctx.enter_context(tc.tile_pool(name="segs", bufs=4))
        outs = ctx.enter_context(tc.tile_pool(name="outs", bufs=1))
        idxs = ctx.enter_context(tc.tile_pool(name="idxs", bufs=4))
        psw = ctx.enter_context(tc.tile_pool(name="psw", bufs=2, space="PSUM"))
        psg = ctx.enter_context(tc.tile_pool(name="psg", bufs=3, space="PSUM"))

        # DMA order matters: small label chunks for the first blocks go in
        # between the big embedding chunks so compute starts immediately.
        enth_t = consts.tile([D, N], edt)
        entl_t = consts.tile([D, N], edt) if USE_FP16 else None
        labwin_t = consts.tile([128, 512 * NBLK], f32)
        labrow_t = consts.tile([128, NBLK], f32)
        gmask_t = consts.tile([128, 16 * G], f32)
        def _lab_chunk(b):
            nc.sync.dma_start(labwin_t[:, 512 * b: 512 * (b + 1)],
                              labwin_d[:, 512 * b: 512 * (b + 1)])

        def _ent_chunk(i):
            nc.sync.dma_start(enth_t[:, 512 * i: 512 * (i + 1)],
                              enth_d[:, 512 * i: 512 * (i + 1)])
            if USE_FP16:
                nc.sync.dma_start(entl_t[:, 512 * i: 512 * (i + 1)],
                                  entl_d[:, 512 * i: 512 * (i + 1)])

        _ent_chunk(0)
        _ent_chunk(1)
        nc.sync.dma_start(labrow_t[:], labrow_d[:])
        _lab_chunk(_BLK_ORDER[0])
        nc.sync.dma_start(gmask_t[:], gmask_d[:])
        for i in range(2, NT):
            _ent_chunk(i)
            if i - 1 < len(_BLK_ORDER):
                _lab_chunk(_BLK_ORDER[i - 1])

        negmax_t = outs.tile([128, NBLK], f32)
        posmin_t = outs.tile([128, NBLK], f32)

        def _mm3(acc_ap, b, t):
            lhsT_h = enth_t[:, 128 * b: 128 * (b + 1)]
            rh = enth_t[:, 512 * t: 512 * (t + 1)]
            if not USE_FP16:
                nc.tensor.matmul(acc_ap, lhsT_h, rh, start=True, stop=True)
                return
            lhsT_l = entl_t[:, 128 * b: 128 * (b + 1)]
            rl = entl_t[:, 512 * t: 512 * (t + 1)]
            nc.tensor.matmul(acc_ap, lhsT_h, rh, start=True, stop=False)
            nc.tensor.matmul(acc_ap, lhsT_h, rl, start=False, stop=False)
            nc.tensor.matmul(acc_ap, lhsT_l, rh, start=False, stop=True)

        def _window_stage(b):
            wtiles = _WT[b]
            slab = slabs.tile([128, N], f32, tag="slab")
            pslab = pslabs.tile([128, 512], f32, tag="pslab")
            seg32 = segs.tile([128, NSEG], f32, tag="seg32")

            lr = labrow_t[:, b: b + 1]
            lw = labwin_t[:, 512 * b: 512 * (b + 1)]
            wneg = masks.tile([128, 512], f32, tag="wneg")
            nc.vector.tensor_scalar(wneg[:], lw, lr, -BIGM,
                                    op0=OP.is_equal, op1=OP.mult)
            wpos = masks.tile([128, 512], f32, tag="wpos")
            nc.vector.tensor_scalar(wpos[:], lw, lr, BIGM,
                                    op0=OP.not_equal, op1=OP.mult)

            accw_map = {}
            for t in wtiles:
                acc = psw.tile([128, 512], f32, tag="accw")
                _mm3(acc[:], b, t)
                accw_map[t] = acc

            for (t, off, l, d) in _wsubpieces(b):
                nc.vector.tensor_tensor(pslab[:, d: d + l],
                                        accw_map[t][:, off: off + l],
                                        wpos[:, d: d + l], op=OP.add)
            for (t, off, l, d) in _wsubpieces(b):
                nc.vector.tensor_tensor(slab[:, 512 * t + off: 512 * t + off + l],
                                        accw_map[t][:, off: off + l],
                                        wneg[:, d: d + l], op=OP.add)
            for (t, off, l) in _wcomplement(b):
                nc.scalar.copy(slab[:, 512 * t + off: 512 * t + off + l],
                               accw_map[t][:, off: off + l])

            mp = posmin_t[:, b: b + 1]
            nc.vector.tensor_reduce(mp, pslab[:], axis=AX.X, op=OP.min)
            pidx = idxs.tile([128, 8], u32, tag="pidx")
            nc.vector.max_index(pidx[:], mp.broadcast_to([128, 8]), pslab[:])
            nc.sync.dma_start(posidx_d[b], pidx[:])

            for t in wtiles:
                nc.vector.tensor_reduce(
                    seg32[:, (512 // G) * t: (512 // G) * (t + 1)],
                    slab[:, 512 * t: 512 * (t + 1)].rearrange(
                        "p (s g) -> p s g", g=G),
                    axis=AX.X, op=OP.max)
            return slab, seg32

        def _rest_stage(b, slab, seg32):
            for run in _groups(b):
                k = len(run)
                t0 = run[0]
                acc = psg.tile([128, 512 * GRP], f32, tag="accg")
                for i, t in enumerate(run):
                    _mm3(acc[:, 512 * i: 512 * (i + 1)], b, t)
                st = slab[:, 512 * t0: 512 * (t0 + k)]
                nc.scalar.copy(st, acc[:, : 512 * k])
                nc.vector.tensor_reduce(
                    seg32[:, (512 // G) * t0: (512 // G) * (t0 + k)],
                    st.rearrange("p (s g) -> p s g", g=G),
                    axis=AX.X, op=OP.max)

            m = negmax_t[:, b: b + 1]
            nc.vector.tensor_reduce(m, seg32[:], axis=AX.X, op=OP.max)

            if b == _DIRECT_BLK:
                s32x = idxs.tile([128, 8], u32, tag="s32x")
                nc.vector.max_index(s32x[:], m.broadcast_to([128, 8]), slab[:])
                nc.sync.dma_start(negseg_d[b], s32x[:])
                return

            s32x = idxs.tile([128, 8], u32, tag="s32x")
            nc.vector.max_index(s32x[:], m.broadcast_to([128, 8]), seg32[:])
            nc.sync.dma_start(negseg_d[b], s32x[:])

            s32f = idxs.tile([128, 1], f32, tag="s32f")
            nc.vector.tensor_copy(s32f[:], s32x[:, 0: 1])
            offf = idxs.tile([128, 1], f32, tag="offf")
            nc.vector.tensor_scalar_mul(offf[:], s32f[:], float(G))
            off16 = idxs.tile([128, 1], u16, tag="off16")
            nc.vector.tensor_copy(off16[:], offf[:])

            gath = idxs.tile([128, 16 * G], f32, tag="gath")
            nc.gpsimd.indirect_copy(
                gath[:].rearrange("p (s g) -> p s g", g=G),
                slab[:].rearrange("p (s g) -> p s g", g=G),
                off16[:], True)
            gadd = idxs.tile([128, 16 * G], f32, tag="gadd")
            nc.vector.tensor_tensor(gadd[:], gath[:], gmask_t[:], op=OP.add)
            g8 = idxs.tile([128, 8], u32, tag="g8")
            nc.vector.max_index(g8[:], m.broadcast_to([128, 8]), gadd[:])
            nc.sync.dma_start(negoff_d[b], g8[:])

        for b in _BLK_ORDER:
            slab, seg32 = _window_stage(b)
            _rest_stage(b, slab, seg32)

        nc.sync.dma_start(negmax_d[:], negmax_t[:])
        nc.sync.dma_start(posmin_d[:], posmin_t[:])

    nc.compile()
    return nc


def _get_nc():
    global _NC
    if _NC is None:
        _NC = _build()
    return _NC


def _prep(embeddings, labels):
    e = np.asarray(embeddings, dtype=np.float32)
    lab = np.asarray(labels).astype(np.int64)
    norms = np.sqrt(np.sum(e * e, axis=1, keepdims=True, dtype=np.float32))
    en = e / np.maximum(norms, np.float32(EPS))

    perm = np.argsort(lab, kind="stable")
    en_s = en[perm]
    lab_s = lab[perm]
    ent = np.ascontiguousarray(en_s.T) * np.float32(SCALE)    # [D, N], scaled
    if USE_FP16:
        enth = ent.astype(np.float16)
        entl = (ent - enth.astype(np.float32)).astype(np.float16)
    else:
        enth, entl = ent, None
    labf = lab_s.astype(np.float32)

    # runs of equal labels in sorted order
    starts = np.zeros(N, dtype=np.int64)
    ends = np.zeros(N, dtype=np.int64)
    bnd = np.flatnonzero(np.concatenate(([1], np.diff(lab_s) != 0, [1])))
    for i in range(len(bnd) - 1):
        starts[bnd[i]:bnd[i + 1]] = bnd[i]
        ends[bnd[i]:bnd[i + 1]] = bnd[i + 1]

    p = np.arange(128)
    gmask = np.full((128, 16, G), np.float32(-3.0e38), dtype=np.float32)
    gmask[p, p % 16, :] = np.float32(0.0)
    gmask = np.ascontiguousarray(gmask.reshape(128, 16 * G))

    in_maps = []
    for c in range(NCORES):
        rot = (np.arange(N) + 1024 * c) % N
        enth_c = np.ascontiguousarray(enth[:, rot])
        entl_c = np.ascontiguousarray(entl[:, rot]) if USE_FP16 else None
        labcol = labf[rot]
        labwin = np.empty((512 * NBLK,), dtype=np.float32)
        for b in range(NBLK):
            w = _wstart(b)
            idx = (w + np.arange(512)) % N
            labwin[512 * b: 512 * (b + 1)] = labcol[idx]
        labwin128 = np.ascontiguousarray(np.broadcast_to(labwin, (128, 512 * NBLK)))
        labrow = np.ascontiguousarray(
            labf[1024 * c: 1024 * (c + 1)].reshape(NBLK, 128).T)

        # safety: every anchor's label run must fit its block window
        rs = starts[1024 * c: 1024 * (c + 1)]
        re = ends[1024 * c: 1024 * (c + 1)]
        a = np.arange(1024)
        blk = a // 128
        lo = 1024 * c + 128 * blk - 192
        hi = 1024 * c + 128 * blk + 320
        if not ((rs >= lo).all() and (re <= hi).all()):
            raise AssertionError("label run exceeds window; layout assumption broken")

        im = {"enth": enth_c, "labwin": labwin128,
              "labrow": labrow, "gmask": gmask}
        if USE_FP16:
            im["entl"] = entl_c
        in_maps.append(im)
    return in_maps, perm, lab_s


def _assemble(results, perm):
    pos_inds = np.empty(N, dtype=np.int64)
    neg_inds = np.empty(N, dtype=np.int64)
    pos_dist = np.empty(N, dtype=np.float32)
    neg_dist = np.empty(N, dtype=np.float32)

    one = np.float32(1.0)
    p = np.arange(128)
    for c in range(NCORES):
        r = results[c]
        negmax = np.asarray(r["negmax"])            # [128, NBLK]
        posmin = np.asarray(r["posmin"])
        negseg = np.asarray(r["negseg"])[:, :, 0]   # [NBLK, 128] first G-seg
        negoff = np.asarray(r["negoff"])[:, :, 0]   # [NBLK, 128] gather-flat idx
        posidx = np.asarray(r["posidx"])[:, :, 0]

        for b in range(NBLK):
            srow = 1024 * c + 128 * b + p                   # sorted row ids
            orow = perm[srow]                               # original rows
            s2i = np.float32(1.0 / SCALE2)
            nd = (one - negmax[:, b] * s2i).astype(np.float32)
            pd = (one - posmin[:, b] * s2i).astype(np.float32)

            if b == _DIRECT_BLK:
                ncol_rot = negseg[b].astype(np.int64)       # direct flat scan
            else:
                w = negoff[b].astype(np.int64) - (p % 16) * G
                if not ((w >= 0).all() and (w < G).all()):
                    raise AssertionError("neg gather decode out of range")
                ncol_rot = negseg[b].astype(np.int64) * G + w

            pf = posidx[b].astype(np.int64)                 # [0, 512)
            pcol_rot = (_wstart(b) + pf) % N

            ncol = perm[(ncol_rot + 1024 * c) % N]
            pcol = perm[(pcol_rot + 1024 * c) % N]
            neg_dist[orow] = nd
            pos_dist[orow] = pd
            neg_inds[orow] = ncol
            pos_inds[orow] = pcol

    anchors = np.arange(N, dtype=np.int32)
    triplets = np.column_stack(
        (anchors, pos_inds.astype(np.int32), neg_inds.astype(np.int32)))
    return triplets, pos_dist, neg_dist


def kernel(embeddings, labels):
    nc = _get_nc()
    in_maps, perm, _ = _prep(embeddings, labels)
    last = None
    for attempt in range(3):
        try:
            res = run_bass_kernel_spmd(nc, in_maps, core_ids=list(range(NCORES)))
            return _assemble(res.results, perm)
        except Exception as e:  # transient device-unrecoverable on fresh NEFFs
            last = e
            import time as _time
            _time.sleep(2.0)
    raise last
